# revision 7
# baseline (speedup 1.0000x reference)
"""Trainium2 Bass kernel v2 for nn_DeformAttn (sparse per-pixel attention).

Computation (per batch b, H=8 heads x 16 ch, S=9 samples, D=16384 pixels):
  qp = Wq@q + bq ; kp = Wk@kv ; vp = Wv@kv + bv
  logits[h,s,d] = sum_{c in h} qp[c,d] * kp[c,s,d] * 0.25
  attn = softmax_s(logits); out[c,d] = sum_s attn[h(c),s,d] * vp[c,s,d]
  (bk dropped: its logit contribution is constant over s, softmax-invariant)

Sharding: 8 cores = batch(4) x spatial-half(2). No collectives.

v2 design vs v1 (281us -> measured below):
  - fp16 inputs/weights/output (host casts); halves DMA and SBUF
  - bkm logit-bias matmul deleted (bk term is constant over s -> cancels
    in softmax); bv folded into the u-product via scalar_tensor_tensor
  - kp computed in 2-bank PSUM pairs, pair-drained by ACT (no bias),
    t = qp*kp as paired 2x DVE TTs (qp broadcast over the pair dim)
  - vp computed JIT in the value phase; u = (vp+bv)*ae_exp via
    scalar_tensor_tensor reading both PSUM operands (no vp drains)
  - normalize-at-end softmax: the expansion masks stream unnormalized
    exp (ready one pipeline stage early, so nothing at the iteration
    start waits on the softmax chain); all 9 u_s identity-accumulate on
    PE into o_ps; tail = one DVE multiply out = o * bcast(1/Z) -> fp16
    SBUF -> DMA (no out drain, no attn materialization)
  - software-pipelined emission: value phase of tile t is interleaved
    instruction-by-instruction with the q/k/logit phase of tile t+1 so
    the in-order PE queue always has ready work; samples 6..8 of t are
    held back to cover t+1's exp->gm->recip latency window
  - PSUM = exactly 8 banks: at(qp,lg,z) 1 | kp pair 2 | vp single x2 |
    ae single x2 | o 1 (rzx borrows a vp rotation slot at the tail)
"""
import os
import sys

for _p in ("/opt/trn_rl_repo", "/root/.axon_site/_ro/trn_rl_repo"):
    if os.path.isdir(_p) and _p not in sys.path:
        sys.path.insert(0, _p)

import numpy as np
from contextlib import ExitStack

import concourse.bass as bass
import concourse.bacc as bacc
import concourse.tile as tile
from concourse import mybir
from concourse.bass_utils import run_bass_kernel_spmd

F32 = mybir.dt.float32
F16 = mybir.dt.float16
AF = mybir.ActivationFunctionType
ALU = mybir.AluOpType

B, C = 4, 128
H, HC, S = 8, 16, 9
FH, FW = 128, 128
D_FULL = FH * FW          # 16384
D_HALF = D_FULL // 2      # 8192 pixels per core
TN = 512                  # pixels per tile
NT = D_HALF // TN         # 16 tiles
SCALE = HC ** -0.5        # 0.25
N_CORES = 8

# fp16 const blob column layout
KM_OFF = 0                   # kmask: S slices of [128, 72]
EM_OFF = KM_OFF + S * 72     # emask: S slices of [72, 128]
GM_OFF = EM_OFF + S * 128    # gmask [72, 8]
IM_OFF = GM_OFF + 8          # identity [128, 128]
B128_OFF = IM_OFF + 128      # b128 [8, 128]
NB = B128_OFF + 128

# f32 const blob: bq | bv
NF = 2

# u-phase routing: samples routed via ACT drains (vp+ae drained, 2x TT)
# instead of direct stt from PSUM.
U_ACT_SAMPLES = frozenset()
T_DIRECT = frozenset()      # kp groups consumed straight from PSUM


def _build_nc(repeat=1):
    nc = bacc.Bacc("TRN2", target_bir_lowering=False, debug=False,
                   num_devices=N_CORES)
    dp = nc.declare_dram_parameter
    q_d = dp("q", [C, D_HALF], F16, isOutput=False)
    kv_d = dp("kv", [C, NT, S, TN], F16, isOutput=False)
    w_d = dp("blob_w", [C, 3 * C], F16, isOutput=False)   # WqT|WkT|WvT
    bb_d = dp("blob_b", [C, NB], F16, isOutput=False)
    bf_d = dp("blob_f", [C, NF], F32, isOutput=False)
    out_d = dp("out", [C, D_HALF], F16, isOutput=True)

    with ExitStack() as ctx:
        tc = ctx.enter_context(tile.TileContext(nc))
        p_const = ctx.enter_context(tc.tile_pool(name="consts", bufs=1))
        p_qin = ctx.enter_context(tc.tile_pool(name="qin", bufs=4))
        p_kvin = ctx.enter_context(tc.tile_pool(name="kvin", bufs=4))
        p_qp = ctx.enter_context(tc.tile_pool(name="qp", bufs=2))
        p_kpbf = ctx.enter_context(tc.tile_pool(name="kpbf", bufs=3))
        p_t = ctx.enter_context(tc.tile_pool(name="t", bufs=3))
        p_sm = ctx.enter_context(tc.tile_pool(name="sm", bufs=2))
        p_u = ctx.enter_context(tc.tile_pool(name="u", bufs=3))
        p_ub = ctx.enter_context(tc.tile_pool(name="ub", bufs=3))
        p_out = ctx.enter_context(tc.tile_pool(name="outp", bufs=6))
        # PSUM: 8 banks exactly
        ps_at = ctx.enter_context(tc.tile_pool(name="psat", bufs=1, space="PSUM"))
        ps_kp = ctx.enter_context(tc.tile_pool(name="pskp", bufs=1, space="PSUM"))
        ps_vp = ctx.enter_context(tc.tile_pool(name="psvp", bufs=2, space="PSUM"))
        ps_ae = ctx.enter_context(tc.tile_pool(name="psae", bufs=2, space="PSUM"))
        ps_o = ctx.enter_context(tc.tile_pool(name="pso", bufs=1, space="PSUM"))

        # ---- constants (one DMA per blob) ----
        w_sb = p_const.tile([C, 3 * C], F16)
        nc.sync.dma_start(w_sb[:], w_d[:])
        bb_sb = p_const.tile([C, NB], F16)
        nc.sync.dma_start(bb_sb[:], bb_d[:])
        bf_sb = p_const.tile([C, NF], F32)
        nc.sync.dma_start(bf_sb[:], bf_d[:])

        # joins: absorb const-DMA semaphores up front
        nc.tensor.ldweights(bb_sb[:, 0:128])
        nc.tensor.ldweights(w_sb[:, 0:128])
        act_join = p_const.tile([C, 1], F32)
        nc.scalar.copy(act_join[:], bf_sb[:, 0:1])
        dve_join = p_const.tile([C, 1], F32)
        nc.vector.tensor_copy(dve_join[:], bf_sb[:, 0:1])

        wq_t = w_sb[:, 0:C]
        wk_t = w_sb[:, C:2 * C]
        wv_t = w_sb[:, 2 * C:3 * C]
        bq_col = bf_sb[:, 0:1]
        bv_col = bf_sb[:, 1:2]
        gm = bb_sb[0:72, GM_OFF:GM_OFF + 8]
        im = bb_sb[:, IM_OFF:IM_OFF + 128]
        b128 = bb_sb[0:8, B128_OFF:B128_OFF + 128]

        def km(s):
            return bb_sb[:, KM_OFF + s * 72:KM_OFF + (s + 1) * 72]

        def em(s):
            return bb_sb[0:72, EM_OFF + s * 128:EM_OFF + (s + 1) * 128]

        def _body():
            # state carried between pipeline stages, keyed by tile index
            st = {}

            def emit_loads(t, split=False):
                q_t = p_qin.tile([C, TN], F16, name="q_t")
                nc.sync.dma_start(q_t[:], q_d[:, t * TN:(t + 1) * TN])
                kv_t = p_kvin.tile([C, S, TN], F16, name="kv_t")
                if split:
                    for s in range(S):
                        nc.sync.dma_start(kv_t[:, s], kv_d[:, t, s])
                else:
                    nc.sync.dma_start(kv_t[:], kv_d[:, t])
                st[t] = {"q": q_t, "kv": kv_t}

            # --- A-phase pieces for tile t (q/k/logit/softmax-front) ---
            def a_kp_mm(t, p):
                d = st[t]
                n = 2 if p < 4 else 1
                kpp = ps_kp.tile([C, 2, TN], F32, name="kp_pp")
                for j in range(n):
                    nc.tensor.matmul(kpp[:, j], wk_t, d["kv"][:, 2 * p + j],
                                     start=True, stop=True)
                d[f"kp{p}"] = kpp

            def a_qp(t):
                d = st[t]
                qp_ps = ps_at.tile([C, TN], F32, tag="at", name="qp_ps")
                nc.tensor.matmul(qp_ps[:], wq_t, d["q"][:], start=True, stop=True)
                qp_bf = p_qp.tile([C, TN], F16, name="qp_bf")
                nc.scalar.activation(qp_bf[:], qp_ps[:], AF.Identity, bias=bq_col)
                d["qp"] = qp_bf
                d["lg"] = ps_at.tile([72, TN], F32, tag="at", name="lg_ps")

            def a_kp_drain_tt(t, p):
                d = st[t]
                n = 2 if p < 4 else 1
                kpp = d.pop(f"kp{p}")
                t_sb = p_t.tile([C, 2, TN], F16, name="t_sb")
                if p in T_DIRECT:
                    # direct from PSUM (one-PSUM-operand rule: qp is SBUF)
                    qp_bc = d["qp"].unsqueeze(1).broadcast_to([C, 2, TN])
                    if n == 2:
                        nc.vector.tensor_mul(t_sb[:], qp_bc, kpp[:])
                    else:
                        nc.vector.tensor_mul(t_sb[:, 0], d["qp"][:], kpp[:, 0])
                elif n == 2:
                    kpb = p_kpbf.tile([C, 2, TN], F16, name="kpb")
                    nc.scalar.copy(
                        kpb.rearrange("c s n -> c (s n)"),
                        kpp.rearrange("c s n -> c (s n)"))
                    qp_bc = d["qp"].unsqueeze(1).broadcast_to([C, 2, TN])
                    nc.vector.tensor_mul(t_sb[:], qp_bc, kpb[:])
                else:
                    kpb = p_kpbf.tile([C, 2, TN], F16, name="kpb")
                    nc.scalar.copy(kpb[:, 0], kpp[:, 0])
                    nc.vector.tensor_mul(t_sb[:, 0], d["qp"][:], kpb[:, 0])
                d[f"t{p}"] = t_sb

            def a_masks(t, p):
                d = st[t]
                n = 2 if p < 4 else 1
                t_sb = d.pop(f"t{p}")
                for j in range(n):
                    s = 2 * p + j
                    nc.tensor.matmul(d["lg"][:], km(s), t_sb[:, j],
                                     start=(s == 0), stop=(s == S - 1))

            def a_softmax_front(t):
                d = st[t]
                exp_sb = p_sm.tile([72, TN], F16, tag="exp", name="exp_sb")
                nc.scalar.activation(exp_sb[:], d.pop("lg")[:], AF.Exp,
                                     scale=SCALE)
                z_ps = ps_at.tile([8, TN], F32, tag="at", name="z_ps")
                nc.tensor.matmul(z_ps[:], gm, exp_sb[:], start=True, stop=True)
                rz_f = p_sm.tile([8, TN], F32, tag="rzf", name="rz_f")
                nc.vector.reciprocal_approx_fast(out=rz_f[:], in_=z_ps[:])
                rz_sb = p_sm.tile([8, TN], F16, tag="rz", name="rz_sb")
                nc.vector.tensor_copy(rz_sb[:], rz_f[:])
                d["exp"] = exp_sb
                d["rz"] = rz_sb

            # --- C-phase pieces for tile t (value phase) ---
            def c_mm(t, s):
                """ae + vp matmuls for sample s."""
                d = st[t]
                aes = ps_ae.tile([C, TN], F32, name="ae_ps")
                vps = ps_vp.tile([C, TN], F32, name="vp_ps")
                nc.tensor.matmul(aes[:], em(s), d["exp"][:],
                                 start=True, stop=True)
                nc.tensor.matmul(vps[:], wv_t, d["kv"][:, s],
                                 start=True, stop=True)
                d[f"ae{s}"] = aes
                d[f"vp{s}"] = vps

            def c_u(t, s):
                """u_s = (vp_s + bv) * ae_s. HW allows only one PSUM operand
                per DVE op, so ae drains to SBUF fp16 via ACT first."""
                d = st[t]
                aes = d.pop(f"ae{s}")
                vps = d.pop(f"vp{s}")
                u = p_u.tile([C, TN], F16, name="u_sb")
                aeb = p_ub.tile([C, TN], F16, tag="aeb", name="aeb")
                nc.scalar.copy(aeb[:], aes[:])
                if s in U_ACT_SAMPLES:
                    vpb = p_ub.tile([C, TN], F16, tag="vpb", name="vpb")
                    nc.scalar.activation(vpb[:], vps[:], AF.Identity,
                                         bias=bv_col)
                    nc.vector.tensor_mul(u[:], vpb[:], aeb[:])
                else:
                    nc.vector.scalar_tensor_tensor(
                        out=u[:], in0=vps[:], scalar=bv_col, in1=aeb[:],
                        op0=ALU.add, op1=ALU.mult)
                d[f"u{s}"] = u

            def c_acc(t, s):
                """PE identity-acc of sample s into o."""
                d = st[t]
                o_ps = d.get("o")
                if o_ps is None:
                    o_ps = ps_o.tile([C, TN], F32, name="o_ps")
                    d["o"] = o_ps
                nc.tensor.matmul(o_ps[:], im, d.pop(f"u{s}")[:],
                                 start=(s == 0), stop=(s == S - 1))

            def c_rzx(t):
                """Broadcast 1/Z to all 128 channel rows: rzx = b128 @ rz."""
                d = st[t]
                rzx_ps = ps_vp.tile([C, TN], F32, name="vp_ps")
                nc.tensor.matmul(rzx_ps[:], b128, d.pop("rz")[:],
                                 start=True, stop=True)
                d["rzx"] = rzx_ps

            def c_tail(t):
                """out = o * rzx; store (rzx drains to SBUF first: the HW
                allows only one PSUM operand per DVE op)."""
                d = st[t]
                rzxb = p_sm.tile([C, TN], F16, tag="rzxb", name="rzxb")
                nc.scalar.copy(rzxb[:], d["rzx"][:])
                out_sb = p_out.tile([C, TN], F16, name="out_sb")
                nc.vector.tensor_mul(out_sb[:], d["o"][:], rzxb[:])
                nc.sync.dma_start(out_d[:, t * TN:(t + 1) * TN], out_sb[:])
                del st[t]

            def emit_A_solo(t):
                """Un-interleaved A phase (prologue)."""
                a_qp(t)
                for p in range(5):
                    a_kp_mm(t, p)
                    a_kp_drain_tt(t, p)
                    a_masks(t, p)
                a_softmax_front(t)

            def emit_iteration(t):
                """Value phase of t interleaved with A(t+1). The value phase
                streams unnormalized exp; 1/Z applies at the tail, so the
                iteration start depends only on exp/rz of the previous
                iteration's softmax-front."""
                last = t + 1 >= NT

                def A(fn, *a):
                    if not last:
                        fn(*a)

                A(a_kp_mm, t + 1, 0)
                c_mm(t, 0)
                c_u(t, 0)
                c_mm(t, 1)
                A(a_qp, t + 1)
                A(a_kp_drain_tt, t + 1, 0)
                c_u(t, 1)
                for p in (1, 2):
                    A(a_kp_mm, t + 1, p)
                    c_mm(t, 2 * p)
                    c_acc(t, 2 * p - 2)
                    c_u(t, 2 * p)
                    c_mm(t, 2 * p + 1)
                    c_acc(t, 2 * p - 1)
                    A(a_masks, t + 1, p - 1)
                    c_u(t, 2 * p + 1)
                    A(a_kp_drain_tt, t + 1, p)
                A(a_kp_mm, t + 1, 3)
                c_acc(t, 4)
                c_acc(t, 5)
                A(a_masks, t + 1, 2)
                A(a_kp_drain_tt, t + 1, 3)
                A(a_kp_mm, t + 1, 4)
                A(a_masks, t + 1, 3)
                A(a_kp_drain_tt, t + 1, 4)
                A(a_masks, t + 1, 4)
                A(a_softmax_front, t + 1)
                c_mm(t, 6)
                c_u(t, 6)
                c_mm(t, 7)
                c_acc(t, 6)
                c_u(t, 7)
                c_mm(t, 8)
                c_u(t, 8)
                c_rzx(t)
                c_acc(t, 7)
                c_acc(t, 8)
                c_tail(t)

            # ---- pipeline ----
            emit_loads(0, split=True)
            emit_loads(1)
            emit_A_solo(0)
            for t in range(NT):
                if t + 2 < NT:
                    emit_loads(t + 2)
                emit_iteration(t)

        if repeat == 1:
            _body()
        else:
            with tc.For_i(0, repeat, 1):
                _body()
    nc.compile()
    return nc


def _make_consts(Wq, bq, Wk, bk, Wv, bv):
    del bk  # softmax-invariant, dropped
    blob_w = np.concatenate(
        [np.ascontiguousarray(Wq.T), np.ascontiguousarray(Wk.T),
         np.ascontiguousarray(Wv.T)], axis=1
    ).astype(np.float16)

    blob_b = np.zeros((C, NB), dtype=np.float16)
    cc = np.arange(C)
    kmv = np.zeros((C, 72), dtype=np.float32)
    for s in range(S):
        kmv[:] = 0.0
        kmv[cc, (cc // HC) * S + s] = 1.0
        blob_b[:, KM_OFF + s * 72:KM_OFF + (s + 1) * 72] = kmv.astype(np.float16)
    emv = np.zeros((72, C), dtype=np.float32)
    for s in range(S):
        emv[:] = 0.0
        emv[(cc // HC) * S + s, cc] = 1.0
        blob_b[0:72, EM_OFF + s * 128:EM_OFF + (s + 1) * 128] = \
            emv.astype(np.float16)
    jj = np.arange(72)
    gmask = np.zeros((72, 8), dtype=np.float32)
    gmask[jj, jj // S] = 1.0
    blob_b[0:72, GM_OFF:GM_OFF + 8] = gmask.astype(np.float16)
    blob_b[:, IM_OFF:IM_OFF + 128] = np.eye(C, dtype=np.float32).astype(np.float16)
    b128v = np.zeros((8, C), dtype=np.float32)
    b128v[cc // HC, cc] = 1.0
    blob_b[0:8, B128_OFF:B128_OFF + 128] = b128v.astype(np.float16)

    blob_f = np.zeros((C, NF), dtype=np.float32)
    blob_f[:, 0] = bq
    blob_f[:, 1] = bv
    return blob_w, blob_b, blob_f


_NC_CACHE = []


def _make_in_maps(q, kv, Wq, bq, Wk, bk, Wv, bv):
    blob_w, blob_b, blob_f = _make_consts(Wq, bq, Wk, bk, Wv, bv)
    q_flat = q.reshape(B, C, D_FULL).astype(np.float16)
    kv_flat = kv.reshape(B, C, S, D_FULL).astype(np.float16)
    in_maps = []
    for core in range(N_CORES):
        b = core // 2
        half = core % 2
        sl = slice(half * D_HALF, (half + 1) * D_HALF)
        q_sh = np.ascontiguousarray(q_flat[b, :, sl])
        kv_sh = np.ascontiguousarray(
            kv_flat[b, :, :, sl].reshape(C, S, NT, TN).transpose(0, 2, 1, 3)
        )                                                  # [C, NT, S, TN]
        in_maps.append({
            "q": q_sh, "kv": kv_sh,
            "blob_w": blob_w, "blob_b": blob_b, "blob_f": blob_f,
        })
    return in_maps


def kernel(q, kv, Wq, bq, Wk, bk, Wv, bv):
    q = np.asarray(q, dtype=np.float32)
    kv = np.asarray(kv, dtype=np.float32)
    args = [np.asarray(a, dtype=np.float32) for a in (Wq, bq, Wk, bk, Wv, bv)]
    in_maps = _make_in_maps(q, kv, *args)

    if not _NC_CACHE:
        _NC_CACHE.append(_build_nc())
    nc = _NC_CACHE[0]
    res = run_bass_kernel_spmd(nc, in_maps, list(range(N_CORES)))

    out = np.empty((B, C, D_FULL), dtype=np.float32)
    for core in range(N_CORES):
        b = core // 2
        half = core % 2
        out[b, :, half * D_HALF:(half + 1) * D_HALF] = \
            res.results[core]["out"].astype(np.float32)
    return out.reshape(B, C, FH, FW)


if __name__ == "__main__":
    rng = np.random.default_rng(0)
    ins = {
        "q": rng.standard_normal((B, C, FH, FW), dtype=np.float32),
        "kv": rng.standard_normal((B, C, S, D_FULL), dtype=np.float32),
        "Wq": rng.standard_normal((C, C), dtype=np.float32) * C ** -0.5,
        "bq": (rng.standard_normal(C) * 0.01).astype(np.float32),
        "Wk": rng.standard_normal((C, C), dtype=np.float32) * C ** -0.5,
        "bk": (rng.standard_normal(C) * 0.01).astype(np.float32),
        "Wv": rng.standard_normal((C, C), dtype=np.float32) * C ** -0.5,
        "bv": (rng.standard_normal(C) * 0.01).astype(np.float32),
    }
    out = kernel(**ins)
    print("ran, out shape", out.shape, "finite:", np.isfinite(out).all())


# revision 9
# speedup vs baseline: 1.8685x; 1.8685x over previous
"""Trainium2 Bass kernel for nn_DeformAttn (sparse per-pixel attention).

Computation (per batch b, H=8 heads x 16 ch, S=9 samples, D=16384 pixels):
  qp = Wq@q + bq ; kp = Wk@kv ; vp = Wv@kv
  logits[h,s,d] = sum_c_in_head (qp+bq)[c,d] * (kp+bk)[c,d] * 0.25
  attn = softmax_s(logits); out[c,d] = sum_s attn[h(c),s,d] * (vp+bv)[c,d]

Sharding: 8 cores = batch(4) x spatial-half(2). No collectives.

Per-core dataflow (16 tiles of 512 pixels), all matmuls N=512:
  - projections as float32r matmuls (1 cyc/row)
  - per-head logit reduction via constant 0/1 bf16 mask matmuls channels->72
    (head,sample) partition rows; bk bias folded in via a bk-mask matmul
    (note sum_c qp'*(kp+bk) = sum qp'kp + sum qp'bk with qp' = qp+bq)
  - softmax over s without max-subtraction (logits ~ +-6): ACT exp,
    sum-over-s + 1/Z broadcast via tiny mask matmuls, reciprocal_approx_fast
  - value phase: PE expands attn [72,N]->[128,N] per sample via bf16 mask
    matmuls; DVE multiplies vs vp, bf16 adds-tree over s; bv added at the
    final ScalarE bias-copy (valid since sum_s attn = 1)
  - DVE/ACT balanced: for s in KP_ACT/AE_ACT the PSUM drain goes through a
    ScalarE bf16 copy so the DVE tensor_tensor runs at 2x; other samples are
    consumed directly from PSUM at 1x.

Hardware constraint that shapes the emission order: every instruction can
carry at most ONE semaphore wait (bf16 matmuls split LDW+MM and so get two
slots; fp32/fp32r self-loading matmuls get one total). The loop is ordered
so each instruction needs at most one NEW semaphore under Tile's
vector-clock subsumption; dummy ldweights/copy "joins" pre-absorb the
constant-blob DMA semaphores, and output staging tiles are dedicated
(bufs=NT) so the store path has no write-after-read waits.
"""
import os
import sys

for _p in ("/opt/trn_rl_repo", "/root/.axon_site/_ro/trn_rl_repo"):
    if os.path.isdir(_p) and _p not in sys.path:
        sys.path.insert(0, _p)

import numpy as np
import ml_dtypes
from contextlib import ExitStack

import concourse.bass as bass
import concourse.bacc as bacc
import concourse.tile as tile
from concourse import mybir
from concourse.bass_utils import run_bass_kernel_spmd

F32 = mybir.dt.float32
F32R = mybir.dt.float32r
BF16 = mybir.dt.float16  # fp16: 10-bit mantissa, same PE/DVE speed as bf16
AF = mybir.ActivationFunctionType

B, C = 4, 128
H, HC, S = 8, 16, 9
FH, FW = 128, 128
D_FULL = FH * FW          # 16384
D_HALF = D_FULL // 2      # 8192 pixels per core
TN = 512                  # pixels per tile
NT = D_HALF // TN         # 16 tiles
SCALE = HC ** -0.5        # 0.25
N_CORES = 8

# samples whose PSUM drain routes through ScalarE (bf16 copy -> DVE 2x TT).
# Must be a prefix {0..k} so the first DVE op of each group carries the one
# allowed ACT wait and later direct-PSUM ops inherit the ACT clock.
KP_ACT = frozenset({0, 1, 2})
AE_ACT = frozenset({0, 1, 2, 3, 4})

# bf16 const blob column layout
KM_OFF = 0                 # kmask: S slices of [128, 72]
BKM_OFF = KM_OFF + S * 72  # bkmask [128, 72]
EM_OFF = BKM_OFF + 72      # emask: S slices of [72, 128]
GM_OFF = EM_OFF + S * 128  # gmask [72, 8]
IM_OFF = GM_OFF + 8        # identity [128, 128]
NB = IM_OFF + 128          # bf16 cols

# f32 const blob column layout: bq | bv | b72 [8, 72]
NF = 2 + 72


def _build_nc(repeat=1):
    nc = bacc.Bacc("TRN2", target_bir_lowering=False, debug=False,
                   num_devices=N_CORES)
    dp = nc.declare_dram_parameter
    q_d = dp("q", [C, D_HALF], F32R, isOutput=False)
    kv_d = dp("kv", [C, NT, S, TN], F32R, isOutput=False)
    wr_d = dp("blob_r", [C, 3 * C], F32R, isOutput=False)   # WqT|WkT|WvT
    bb_d = dp("blob_b", [C, NB], BF16, isOutput=False)
    bf_d = dp("blob_f", [C, NF], F32, isOutput=False)
    out_d = dp("out", [C, D_HALF], F32, isOutput=True)

    with ExitStack() as ctx:
        tc = ctx.enter_context(tile.TileContext(nc))
        p_const = ctx.enter_context(tc.tile_pool(name="consts", bufs=1))
        p_qin = ctx.enter_context(tc.tile_pool(name="qin", bufs=4))
        p_kvin = ctx.enter_context(tc.tile_pool(name="kvin", bufs=4))
        p_qp = ctx.enter_context(tc.tile_pool(name="qp", bufs=3))
        p_kpbf = ctx.enter_context(tc.tile_pool(name="kpbf", bufs=6))
        p_t = ctx.enter_context(tc.tile_pool(name="t", bufs=6))
        p_vp = ctx.enter_context(tc.tile_pool(name="vp", bufs=3))
        p_sm = ctx.enter_context(tc.tile_pool(name="sm", bufs=3))
        p_aebf = ctx.enter_context(tc.tile_pool(name="aebf", bufs=6))
        p_u = ctx.enter_context(tc.tile_pool(name="u", bufs=6))
        p_out = ctx.enter_context(tc.tile_pool(name="outp", bufs=NT))
        ps_kp = ctx.enter_context(tc.tile_pool(name="pskp", bufs=2, space="PSUM"))
        ps_vp = ctx.enter_context(tc.tile_pool(name="psvp", bufs=2, space="PSUM"))
        ps_at = ctx.enter_context(tc.tile_pool(name="psat", bufs=1, space="PSUM"))
        ps_ae = ctx.enter_context(tc.tile_pool(name="psae", bufs=2, space="PSUM"))
        ps_o = ctx.enter_context(tc.tile_pool(name="pso", bufs=1, space="PSUM"))

        # ---- constants (one DMA per blob) ----
        wr_sb = p_const.tile([C, 3 * C], F32R)
        nc.sync.dma_start(wr_sb[:], wr_d[:])
        bb_sb = p_const.tile([C, NB], BF16)
        nc.sync.dma_start(bb_sb[:], bb_d[:])
        bf_sb = p_const.tile([C, NF], F32)
        nc.sync.dma_start(bf_sb[:], bf_d[:])

        # joins: let PE/ACT observe each const-DMA queue up front so later
        # 1-wait-limited instructions only wait on their streaming operand
        nc.tensor.ldweights(bb_sb[:, 0:128])
        nc.tensor.ldweights(wr_sb[:, 0:64].bitcast(BF16))
        nc.tensor.ldweights(bf_sb[:, 0:64].bitcast(BF16))
        act_join = p_const.tile([C, 1], F32)
        nc.scalar.copy(act_join[:], bf_sb[:, 0:1])
        dve_join = p_const.tile([C, 1], F32)
        nc.vector.tensor_copy(dve_join[:], bf_sb[:, 0:1])

        wq_t = wr_sb[:, 0:C]
        wk_t = wr_sb[:, C:2 * C]
        wv_t = wr_sb[:, 2 * C:3 * C]
        bq_col = bf_sb[:, 0:1]
        bv_col = bf_sb[:, 1:2]
        b72 = bf_sb[0:8, 2:2 + 72]
        bkm = bb_sb[:, BKM_OFF:BKM_OFF + 72]
        gm = bb_sb[0:72, GM_OFF:GM_OFF + 8]
        im = bb_sb[:, IM_OFF:IM_OFF + 128]

        def _body():
          for t in range(NT):
              # ---- loads ----
              q_t = p_qin.tile([C, TN], F32R)
              nc.sync.dma_start(q_t[:], q_d[:, t * TN:(t + 1) * TN])
              kv_t = p_kvin.tile([C, S, TN], F32R)
              nc.sync.dma_start(kv_t[:], kv_d[:, t])

              # ---- q projection + bias (fp32 and bf16 twins) ----
              qp_ps = ps_kp.tile([C, TN], F32, tag="kp_ps")
              nc.tensor.matmul(qp_ps[:], wq_t, q_t[:], start=True, stop=True)
              qp_bf = p_qp.tile([C, TN], BF16, tag="qpb")
              nc.scalar.activation(qp_bf[:], qp_ps[:], AF.Identity, bias=bq_col)

              # ---- k projections, t = qp*kp, logit mask-matmuls ----
              lg_ps = ps_at.tile([72, TN], F32, tag="at")
              kp_list = []
              for s in range(min(2, S)):
                  kp_ps = ps_kp.tile([C, TN], F32)
                  nc.tensor.matmul(kp_ps[:], wk_t, kv_t[:, s], start=True, stop=True)
                  kp_list.append(kp_ps)
              vp_bf = p_vp.tile([C, S, TN], BF16)
              for s in range(S):
                  if s + 2 < S:
                      kp_ps = ps_kp.tile([C, TN], F32)
                      nc.tensor.matmul(kp_ps[:], wk_t, kv_t[:, s + 2],
                                       start=True, stop=True)
                      kp_list.append(kp_ps)
                  kp_ps = kp_list[s]
                  t_sb = p_t.tile([C, TN], BF16)
                  if s in KP_ACT:
                      kp_bf = p_kpbf.tile([C, TN], BF16)
                      nc.scalar.copy(kp_bf[:], kp_ps[:])
                      nc.vector.tensor_mul(t_sb[:], qp_bf[:], kp_bf[:])
                  else:
                      nc.vector.tensor_mul(t_sb[:], qp_bf[:], kp_ps[:])
                  nc.tensor.matmul(
                      lg_ps[:], bb_sb[:, KM_OFF + s * 72:KM_OFF + (s + 1) * 72],
                      t_sb[:], start=(s == 0), stop=(s == S - 1),
                  )
                  # interleave v projections + drains: keeps ScalarE dense
                  # during the t-mult phase (vp 0..5 here, 6..8 as PE filler
                  # for the softmax latency chain below)
                  if s < 6:
                      vp_ps = ps_vp.tile([C, TN], F32)
                      nc.tensor.matmul(vp_ps[:], wv_t, kv_t[:, s],
                                       start=True, stop=True)
                      nc.scalar.copy(vp_bf[:, s], vp_ps[:])
              # bk logit bias dropped: constant over s -> cancels in softmax

              # ---- softmax over s (no max-subtraction; logits bounded) ----
              exp_sb = p_sm.tile([72, TN], BF16, tag="exp")
              nc.scalar.activation(exp_sb[:], lg_ps[:], AF.Exp, scale=SCALE)

              z_ps = ps_at.tile([8, TN], F32, tag="at")
              nc.tensor.matmul(z_ps[:], gm, exp_sb[:], start=True, stop=True)

              for s in range(6, S):
                  vp_ps = ps_vp.tile([C, TN], F32)
                  nc.tensor.matmul(vp_ps[:], wv_t, kv_t[:, s], start=True, stop=True)
                  if s == 8:
                      nc.vector.tensor_copy(vp_bf[:, s], vp_ps[:])
                  else:
                      nc.scalar.copy(vp_bf[:, s], vp_ps[:])

              rz_sb = p_sm.tile([8, TN], F32, tag="rz")
              nc.vector.reciprocal_approx_fast(out=rz_sb[:], in_=z_ps[:])
              zb_ps = ps_at.tile([72, TN], F32, tag="at")
              nc.tensor.matmul(zb_ps[:], b72, rz_sb[:], start=True, stop=True)
              # join: absorb the ACT(exp) wait so attn below needs only PE(zb)
              ej_sb = p_sm.tile([8, 1], BF16, tag="ej")
              nc.vector.tensor_copy(ej_sb[:], exp_sb[0:8, 0:1])
              attn_sb = p_sm.tile([72, TN], BF16, tag="attn")
              nc.vector.tensor_mul(attn_sb[:], exp_sb[:], zb_ps[:])

              # ---- expand attn per sample; u = vp * attn_expanded;
              # sum over s on PE via identity-matmul accumulation ----
              o_ps = ps_o.tile([C, TN], F32)
              for s in range(S):
                  ae_ps = ps_ae.tile([C, TN], F32)
                  nc.tensor.matmul(
                      ae_ps[:], bb_sb[0:72, EM_OFF + s * 128:EM_OFF + (s + 1) * 128],
                      attn_sb[:], start=True, stop=True,
                  )
                  u_sb = p_u.tile([C, TN], BF16)
                  if s in AE_ACT:
                      ae_bf = p_aebf.tile([C, TN], BF16)
                      nc.scalar.copy(ae_bf[:], ae_ps[:])
                      nc.vector.tensor_mul(u_sb[:], vp_bf[:, s], ae_bf[:])
                  else:
                      nc.vector.tensor_mul(u_sb[:], vp_bf[:, s], ae_ps[:])
                  nc.tensor.matmul(o_ps[:], im, u_sb[:],
                                   start=(s == 0), stop=(s == S - 1))

              out_sb = p_out.tile([C, TN], F32)
              nc.scalar.activation(out_sb[:], o_ps[:], AF.Identity, bias=bv_col)
              nc.sync.dma_start(out_d[:, t * TN:(t + 1) * TN], out_sb[:])
        if repeat == 1:
            _body()
        else:
            with tc.For_i(0, repeat, 1):
                _body()
    nc.compile()
    return nc


def _make_consts(Wq, bq, Wk, bk, Wv, bv):
    bf = np.float16
    blob_r = np.concatenate(
        [np.ascontiguousarray(Wq.T), np.ascontiguousarray(Wk.T),
         np.ascontiguousarray(Wv.T)], axis=1
    ).astype(np.float32)

    blob_b = np.zeros((C, NB), dtype=bf)
    cc = np.arange(C)
    km = np.zeros((C, 72), dtype=np.float32)
    for s in range(S):
        km[:] = 0.0
        km[cc, (cc // HC) * S + s] = 1.0
        blob_b[:, KM_OFF + s * 72:KM_OFF + (s + 1) * 72] = km.astype(bf)
    bkm = np.zeros((C, 72), dtype=np.float32)
    for s in range(S):
        bkm[cc, (cc // HC) * S + s] = bk
    blob_b[:, BKM_OFF:BKM_OFF + 72] = bkm.astype(bf)
    mm = np.arange(C)
    em = np.zeros((72, C), dtype=np.float32)
    for s in range(S):
        em[:] = 0.0
        em[(mm // HC) * S + s, mm] = 1.0
        blob_b[0:72, EM_OFF + s * 128:EM_OFF + (s + 1) * 128] = em.astype(bf)
    gmask = np.zeros((72, 8), dtype=np.float32)
    jj = np.arange(72)
    gmask[jj, jj // S] = 1.0
    blob_b[0:72, GM_OFF:GM_OFF + 8] = gmask.astype(bf)
    blob_b[:, IM_OFF:IM_OFF + 128] = np.eye(C, dtype=np.float32).astype(bf)

    blob_f = np.zeros((C, NF), dtype=np.float32)
    blob_f[:, 0] = bq
    blob_f[:, 1] = bv
    b72 = np.zeros((8, 72), dtype=np.float32)
    b72[jj // S, jj] = 1.0
    blob_f[0:8, 2:2 + 72] = b72
    return blob_r, blob_b, blob_f


_NC_CACHE = []


def _make_in_maps(q, kv, Wq, bq, Wk, bk, Wv, bv):
    blob_r, blob_b, blob_f = _make_consts(Wq, bq, Wk, bk, Wv, bv)
    q_flat = q.reshape(B, C, D_FULL)
    kv_flat = kv.reshape(B, C, S, D_FULL)
    in_maps = []
    for core in range(N_CORES):
        b = core // 2
        half = core % 2
        sl = slice(half * D_HALF, (half + 1) * D_HALF)
        q_sh = np.ascontiguousarray(q_flat[b, :, sl])
        kv_sh = np.ascontiguousarray(
            kv_flat[b, :, :, sl].reshape(C, S, NT, TN).transpose(0, 2, 1, 3)
        )                                                  # [C, NT, S, TN]
        in_maps.append({
            "q": q_sh, "kv": kv_sh,
            "blob_r": blob_r, "blob_b": blob_b, "blob_f": blob_f,
        })
    return in_maps


def kernel(q, kv, Wq, bq, Wk, bk, Wv, bv):
    q = np.asarray(q, dtype=np.float32)
    kv = np.asarray(kv, dtype=np.float32)
    args = [np.asarray(a, dtype=np.float32) for a in (Wq, bq, Wk, bk, Wv, bv)]
    in_maps = _make_in_maps(q, kv, *args)

    if not _NC_CACHE:
        _NC_CACHE.append(_build_nc())
    nc = _NC_CACHE[0]
    res = run_bass_kernel_spmd(nc, in_maps, list(range(N_CORES)))

    out = np.empty((B, C, D_FULL), dtype=np.float32)
    for core in range(N_CORES):
        b = core // 2
        half = core % 2
        out[b, :, half * D_HALF:(half + 1) * D_HALF] = res.results[core]["out"]
    return out.reshape(B, C, FH, FW)


if __name__ == "__main__":
    rng = np.random.default_rng(0)
    ins = {
        "q": rng.standard_normal((B, C, FH, FW), dtype=np.float32),
        "kv": rng.standard_normal((B, C, S, D_FULL), dtype=np.float32),
        "Wq": rng.standard_normal((C, C), dtype=np.float32) * C ** -0.5,
        "bq": (rng.standard_normal(C) * 0.01).astype(np.float32),
        "Wk": rng.standard_normal((C, C), dtype=np.float32) * C ** -0.5,
        "bk": (rng.standard_normal(C) * 0.01).astype(np.float32),
        "Wv": rng.standard_normal((C, C), dtype=np.float32) * C ** -0.5,
        "bv": (rng.standard_normal(C) * 0.01).astype(np.float32),
    }
    out = kernel(**ins)
    print("ran, out shape", out.shape, "finite:", np.isfinite(out).all())

